# revision 56
# baseline (speedup 1.0000x reference)
# Trainium2 Bass kernel for nn_MultiHeadAttention_29154238005976 (ACAT-style conv-augmented MHA).
#
# Reference computation (B=4, L=1024, D=1024, H=16, DK=64):
#   q/k/v projections; q,k augmented by a "scrambled" depthwise-dense conv
#   (torch-style raw reshapes (b,h,l,dk)->(b, h*dk, l) scramble time/channels);
#   softmax attention per head; output projection.
#
# Sharding: 8 cores = 4 batches x 2 head-halves. All cores run an IDENTICAL
# program; per-core differences are pushed into host-side data permutations:
#   - time permutation sigma(l) = l XOR 8g applied to Q/K/V rows (g = head-half)
#   - channel permutation pi(c) = c XOR 512g applied to WQ/WK columns and to
#     conv kernel rows+columns
#   - conv "time" halves then both map to program range l2' in [0,512)
#   - boundary sig columns get a per-core 0/1 mask (true zero-padding vs
#     wrapped real data)
# Host gathers the 8 partial outputs, un-permutes rows, sums batch pairs, +bfc.
#
# Scheduling notes (cost-model driven):
#   - PE engine needs ~198us of matmul time; everything else is arranged to
#     keep PE gap-free.
#   - The hardware DGE serializes at ~630ns per DMA *instruction*, so DMAs are
#     merged into few wide access patterns (whole-cb projection staging, 2-DMA
#     sig scramble per half, per-quarter conv weights, per-lb output rows).
#   - dm-outer projection groups: the first matmul needs only one 256KB chunk.
#   - Residual transposes are taken from the projection staging tiles, so sig
#     stays 640 cols for both filter lengths and conv eviction is add-only.
#   - Attention is software-pipelined (scores of group g+1 issued before ctx
#     of group g) with the fc matmuls interleaved per qb-half.
#   - Output is written bf16 and upconverted on the host.
import numpy as np
import ml_dtypes

import concourse.bass as bass
import concourse.mybir as mybir
import concourse.tile as tile
from concourse import bacc
from concourse.masks import make_identity

bf16 = ml_dtypes.bfloat16
F32 = mybir.dt.float32
BF16 = mybir.dt.bfloat16
AF = mybir.ActivationFunctionType
OP = mybir.AluOpType

B, L, DM, H, DK = 4, 1024, 1024, 16, 64
FMAX = 4
FILTER_LENGTHS = (2, 4)
N_CORES = 8
SIGW = 640

_CACHE = {}


# ----------------------------------------------------------------------------
# program builder
# ----------------------------------------------------------------------------
def _build(flen: int, zero_bias: bool = False, upto: str = "all") -> bass.Bass:
    nc = bacc.Bacc("TRN2", target_bir_lowering=False, debug=False)

    def din(name, shape, dt):
        return nc.dram_tensor(name, list(shape), dt, kind="ExternalInput").ap()

    Qt_d = din("Qt", (DM, L), BF16)
    Kt_d = din("Kt", (DM, L), BF16)
    Vt_d = din("Vt", (DM, L), BF16)
    Wq_d = din("WQ", (DM, DM), BF16)
    Wk_d = din("WK", (DM, DM), BF16)
    Wv_d = din("WV", (DM, 512), BF16)
    Wfc_d = din("Wfc", (512, DM), BF16)
    cwq_d = din("cwq", (8, 128, flen * 1024), BF16)
    cwk_d = din("cwk", (8, 128, flen * 1024), BF16)
    if not zero_bias:
        bQn_d = din("bQn", (1, DM), BF16)
        bKn_d = din("bKn", (1, DM), BF16)
        bVn_d = din("bVn", (1, 512), BF16)
    em_d = din("emask", (128, 4), F32)
    out_d = nc.dram_tensor("out", [L, DM], BF16, kind="ExternalOutput").ap()

    CW_BUFS = 3 if flen == 2 else 2
    PT_BUFS = (12 if zero_bias else 10) if flen == 2 else 6

    with tile.TileContext(nc) as tc:
        sb = tc.alloc_tile_pool(name="sb", bufs=1)
        dr = tc.alloc_tile_pool(name="dr", bufs=1, space="DRAM")
        psA = tc.alloc_tile_pool(name="psA", bufs=1, space="PSUM")

        # ---- big loads, interleaved in first-use order ------------------
        Qt_sb = sb.tile([128, 8, L], BF16, tag="qkvt", bufs=2, name="Qt_sb")
        Wq_sb = sb.tile([128, 8, DM], BF16, name="Wq_sb")
        qtv = Qt_d.rearrange("(a p) c -> p a c", p=128)
        wqv = Wq_d.rearrange("(a p) c -> p a c", p=128)
        # first chunk split in half so the very first matmuls start sooner
        nc.sync.dma_start(Qt_sb[:, 0, 0:512], qtv[:, 0, 0:512])
        nc.sync.dma_start(Wq_sb[:, 0, 0:512], wqv[:, 0, 0:512])
        nc.sync.dma_start(Qt_sb[:, 0, 512:1024], qtv[:, 0, 512:1024])
        nc.sync.dma_start(Wq_sb[:, 0, 512:1024], wqv[:, 0, 512:1024])
        for a in range(1, 8):
            nc.sync.dma_start(Qt_sb[:, a], qtv[:, a])
            nc.sync.dma_start(Wq_sb[:, a], wqv[:, a])

        if not zero_bias:
            bQn_sb = sb.tile([1, DM], BF16, name="bQn_sb")
            nc.sync.dma_start(bQn_sb, bQn_d)
            bKn_sb = sb.tile([1, DM], BF16, name="bKn_sb")
            nc.sync.dma_start(bKn_sb, bKn_d)
            bVn_sb = sb.tile([1, 512], BF16, name="bVn_sb")
            nc.sync.dma_start(bVn_sb, bVn_d)
        else:
            bQn_sb = bKn_sb = bVn_sb = None
        em_sb = sb.tile([128, 4], F32, name="em_sb")
        nc.sync.dma_start(em_sb, em_d)

        def load_chunked2(t, dram):
            dv = dram.rearrange("(a p) c -> p a c", p=128)
            for a in range(0, 8, 2):
                nc.sync.dma_start(t[:, a:a + 2], dv[:, a:a + 2])

        Wk_sb = sb.tile([128, 8, DM], BF16, name="Wk_sb")
        load_chunked2(Wk_sb, Wk_d)
        Kt_sb = sb.tile([128, 8, L], BF16, tag="qkvt", bufs=2, name="Kt_sb")
        load_chunked2(Kt_sb, Kt_d)

        ones1_sb = sb.tile([1, 128], BF16, name="ones1_sb")
        nc.vector.memset(ones1_sb, 1.0)
        ident_sb = sb.tile([128, 128], BF16, name="ident_sb")
        make_identity(nc, ident_sb)

        qT_sb = sb.tile([128, 4, L], BF16, name="qT_sb")
        kT_sb = sb.tile([128, 4, L], BF16, name="kT_sb")
        ctxT_sb = sb.tile([128, 4, L], BF16, name="ctxT_sb")
        sigq_sb = sb.tile([128, 8, SIGW], BF16, name="sigq_sb")
        sigk_sb = sb.tile([128, 8, SIGW], BF16, name="sigk_sb")
        vpa_sb = sb.tile([128, 8, 520], BF16, name="vpa_sb")
        vpa_r = vpa_sb.rearrange("p lb (hh c) -> p lb hh c", c=65)
        nc.vector.memset(vpa_r[:, :, :, 64], 1.0)  # the denominator "ones" column

        qp_d = dr.tile([L, DM], BF16, name="qp_d")
        kp_d = dr.tile([L, DM], BF16, name="kp_d")

        # ---- phase helpers ---------------------------------------------
        def proj(Xt_sb, W_sb, bn_sb, x_d, T_sb, sig_sb, pfx):
            """x = X @ W + b staged to DRAM (for the sig scramble), with the
            own-half (cb=0) residual transposed on the fly into T_sb.
            dm-outer groups of 2 psums so the first matmul only needs one
            input chunk; evictions are pipelined one group behind; each cb
            half is staged in one SBUF tile, written with a single DMA, and
            its sig slabs are scrambled back in immediately."""
            xdv = x_d.rearrange("(lb p) c -> p lb c", p=128)
            sts = {}

            def evict(p):
                cb, lbs, pss = p
                st = sts[cb]
                for lb, ps in zip(lbs, pss):
                    nc.vector.tensor_copy(st[:, lb], ps)
                    if cb == 0:
                        trp = psA.tile([128, 512], BF16, tag="tr", bufs=3,
                                       name=f"tr_{pfx}_{lb}")
                        for ct in range(4):
                            nc.tensor.matmul(
                                trp[:, ct * 128:ct * 128 + 128],
                                st[:, lb, ct * 128:ct * 128 + 128],
                                ident_sb, is_transpose=True,
                                start=(ct == 0), stop=(ct == 3))
                        # on DVE: Act is busy issuing the sig DMA chain and
                        # GPSIMD cannot read PSUM
                        nc.vector.tensor_copy(
                            T_sb[:, :, lb * 128:lb * 128 + 128],
                            trp.rearrange("p (ct r) -> p ct r", ct=4))
                if lbs[-1] == 7:
                    nc.scalar.dma_start(xdv[:, :, cb * 512:cb * 512 + 512], st)
                    scramble_half(x_d, sig_sb, cb)

            pend = None
            for cb in range(2):
                sts[cb] = sb.tile([128, 8, 512], BF16, tag="stage", bufs=2,
                                  name=f"st_{pfx}_{cb}")
                for g in range(4):
                    lbs = [2 * g, 2 * g + 1]
                    pss = []
                    for lb in lbs:
                        ps = psA.tile([128, 512], F32, tag="A", bufs=5,
                                      name=f"ps_{pfx}_{cb}_{lb}")
                        if not zero_bias:
                            nc.tensor.matmul(ps, ones1_sb[0:1, :],
                                             bn_sb[0:1, cb * 512:cb * 512 + 512],
                                             start=True, stop=False)
                        pss.append(ps)
                    for dm in range(8):
                        for i, lb in enumerate(lbs):
                            nc.tensor.matmul(
                                pss[i],
                                Xt_sb[:, dm, lb * 128:lb * 128 + 128],
                                W_sb[:, dm, cb * 512:cb * 512 + 512],
                                start=(zero_bias and dm == 0), stop=(dm == 7))
                    if pend is not None:
                        evict(pend)
                    pend = (cb, lbs, pss)
            evict(pend)

        def scramble_half(x_d, sig_sb, cb):
            """sig[64*hib + il, dt, cols] <- qp[16*il + jh, 128*dt + 64*hib + jl]
            for the dt slabs derived from channel half cb (dt in [4cb, 4cb+4)).
            sig cols [0,64) = j in [960,1024); cols [64,640) = j in [0,576)."""
            # on the Pool queue: keeps the DVE/Act queues free of the
            # sig-load dependency chain (head-of-line blocking)
            xr = x_d.rearrange("(il jh) (dt hib jl) -> hib il dt jh jl",
                               jh=16, hib=2, jl=64)
            dts = slice(4 * cb, 4 * cb + 4)
            for hib in range(2):
                for dt in range(4 * cb, 4 * cb + 4):
                    dst = sig_sb[64 * hib:64 * hib + 64, dt, 64:64 + 64 * 9]
                    nc.scalar.dma_start(
                        dst.rearrange("p (jh jl) -> p jh jl", jl=64),
                        xr[hib, :, dt, 0:9, :])
                nc.scalar.dma_start(sig_sb[64 * hib:64 * hib + 64, dts, 0:64],
                                    xr[hib, :, dts, 15, :])
            # mask the wrap/pad boundary columns (j' = -2,-1[,512,513])
            nc.gpsimd.tensor_tensor(
                sig_sb[:, dts, 62:64], sig_sb[:, dts, 62:64],
                em_sb[:, None, 0:2].to_broadcast((128, 4, 2)), OP.mult)
            if flen == 4:
                nc.gpsimd.tensor_tensor(
                    sig_sb[:, dts, 576:578], sig_sb[:, dts, 576:578],
                    em_sb[:, None, 2:4].to_broadcast((128, 4, 2)), OP.mult)

        def load_cw(cw_d, pfx):
            """One DMA per o'-quarter of the conv weights."""
            cwv = cw_d.rearrange("dt p x -> p dt x")
            tiles = []
            for quarter in range(4):
                t = sb.tile([128, 8, flen * 256], BF16, tag="cw", bufs=CW_BUFS,
                            name=f"cw_{pfx}_{quarter}")
                nc.sync.dma_start(
                    t, cwv[:, :, quarter * flen * 256:(quarter + 1) * flen * 256])
                tiles.append(t)
            return tiles

        def conv(sig_sb, cw_tiles, T_sb, pfx):
            """T[c,l] += conv output, scrambled back into head-transposed tiles.
            conv psum tile s: partition o' = 128 s + 64 ph + k, free l2' = 64 hl + m."""
            Tr = T_sb.rearrange("p q (m r) -> p q m r", r=16)
            for quarter in range(4):
                pss = [psA.tile([128, 512], F32, tag="A", bufs=5,
                                name=f"cps_{pfx}_{quarter}_{si}") for si in range(2)]
                cwt = cw_tiles[quarter]
                for dt in range(8):
                    for si in range(2):
                        for f in range(flen):
                            nc.tensor.matmul(
                                pss[si],
                                cwt[:, dt, f * 256 + si * 128:f * 256 + si * 128 + 128],
                                sig_sb[:, dt, 62 + f:62 + f + 512],
                                start=(dt == 0 and f == 0),
                                stop=(dt == 7 and f == flen - 1))
                for si in range(2):
                    sblk = 2 * quarter + si
                    ps = pss[si].rearrange("p (q h m) -> p q h m", q=4, h=2)
                    for ph in range(2):
                        for pe in range(2):
                            dst = Tr[64 * pe:64 * pe + 64, :, :, 2 * sblk + ph]
                            nc.vector.tensor_tensor(
                                dst, ps[64 * ph:64 * ph + 64, :, pe, :], dst,
                                OP.add)

        # ---- phase sequence --------------------------------------------
        PHASES = ["q", "k", "convq", "convk", "vp", "attn", "all"]
        lim = PHASES.index(upto)

        proj(Qt_sb, Wq_sb, bQn_sb, qp_d, qT_sb, sigq_sb, "q")

        if lim >= 1:
            proj(Kt_sb, Wk_sb, bKn_sb, kp_d, kT_sb, sigk_sb, "k")

        # conv weight prefetch + late big loads. Order matters: the cw ring
        # (bufs=3) stalls the SP queue as conv consumes tiles, which keeps
        # Vt/Wv/Wfc transfers from competing with the sig scramble reads.
        cwq_tiles = load_cw(cwq_d, "q")
        cwk_tiles = load_cw(cwk_d, "k")
        Vt_sb = sb.tile([128, 8, L], BF16, tag="qkvt", bufs=2, name="Vt_sb")
        load_chunked2(Vt_sb, Vt_d)
        Wv_sb = sb.tile([128, 8, 512], BF16, name="Wv_sb")
        nc.sync.dma_start(Wv_sb, Wv_d.rearrange("(a p) c -> p a c", p=128))
        Wfc_sb = sb.tile([128, 4, DM], BF16, name="Wfc_sb")
        nc.sync.dma_start(Wfc_sb, Wfc_d.rearrange("(t p) c -> p t c", p=128))

        # ---- v-projection + attention + fc (software-pipelined) ---------
        odv = out_d.rearrange("(lb p) c -> p lb c", p=128)
        psB = None

        def vp_unit(lb):
            if psB is None:
                ps = psA.tile([128, 512], F32, tag="A", bufs=5,
                              name=f"psv_{lb}")
            else:
                ps = psB.tile([128, 512], F32, tag="cfc", bufs=4,
                              name=f"psv_{lb}")
            if not zero_bias:
                nc.tensor.matmul(ps, ones1_sb[0:1, :], bVn_sb[0:1, :],
                                 start=True, stop=False)
            for dm in range(8):
                nc.tensor.matmul(ps, Vt_sb[:, dm, lb * 128:lb * 128 + 128],
                                 Wv_sb[:, dm, :],
                                 start=(zero_bias and dm == 0), stop=(dm == 7))
            nc.vector.tensor_copy(
                vpa_r[:, lb, :, 0:64],
                ps.rearrange("p (hh c) -> p hh c", hh=8))

        def scores_half(qb, p4, half, pt_tiles):
            """QK^T + exp for two kt2 blocks of one (qb, p4) group."""
            for kt2 in (0, 1) if half == 0 else (2, 3):
                for pe in range(2):
                    ps_st = psB.tile([128, 1024], F32, tag="st", bufs=2,
                                     name=f"st_{qb}_{p4}_{kt2}_{pe}")
                    for h in range(2):
                        kt = 2 * kt2 + h
                        nc.tensor.matmul(
                            ps_st[:, 512 * h:512 * h + 512],
                            kT_sb[64 * pe:64 * pe + 64, p4, kt * 128:kt * 128 + 128],
                            qT_sb[64 * pe:64 * pe + 64, p4, qb * 512:qb * 512 + 512],
                            start=True, stop=True, tile_position=(64 * pe, 0))
                    pt = sb.tile([128, 1024], BF16, tag="pt", bufs=PT_BUFS,
                                 name=f"pt_{qb}_{p4}_{kt2}_{pe}")
                    nc.scalar.activation(pt, ps_st, AF.Exp, scale=0.125)
                    pt_tiles[pe][2 * kt2] = pt[:, 0:512]
                    pt_tiles[pe][2 * kt2 + 1] = pt[:, 512:1024]

        def ctx_phase(qb, p4, pt_tiles, tail=False):
            for pe in range(2):
                hl = 2 * p4 + pe
                # ctx accumulates in rows [0,65); the reciprocal broadcast
                # reuses rows [64,128) of the same bank after the denominator
                # row has been consumed
                ps_ctx = psB.tile([128, 512], F32, tag="cfc", bufs=4,
                                  name=f"ctx_{qb}_{p4}_{pe}")
                for kt in range(8):
                    nc.tensor.matmul(
                        ps_ctx[0:65, :], vpa_sb[:, kt, 65 * hl:65 * hl + 65],
                        pt_tiles[pe][kt], start=(kt == 0), stop=(kt == 7))
                rcb = sb.tile([1, 512], BF16, tag="recipb", bufs=2,
                              name=f"rcb_{qb}_{p4}_{pe}")
                if tail:
                    # shortcut for the final groups (where the divide chain is
                    # exposed): stage the unnormalized ctx rows on the
                    # now-idle Act engine in parallel with the reciprocal
                    un = sb.tile([64, 512], F32, tag="bcs", bufs=2,
                                 name=f"un_{qb}_{p4}_{pe}")
                    nc.scalar.activation(un, ps_ctx[0:64, :], AF.Copy)
                with nc.allow_low_precision(reason="softmax denominators are "
                                            "O(100); bf16 reciprocal is ample"):
                    nc.vector.reciprocal(rcb, ps_ctx[64:65, :])
                nc.tensor.matmul(ps_ctx[64:128, :], ones1_sb[0:1, 0:64], rcb,
                                 start=True, stop=True)
                dst = ctxT_sb[64 * pe:64 * pe + 64, p4, qb * 512:qb * 512 + 512]
                if tail:
                    nc.vector.tensor_tensor(dst, un, ps_ctx[64:128, :], OP.mult)
                else:
                    bc_sb = sb.tile([64, 512], F32, tag="bcs", bufs=2,
                                    name=f"bcs_{qb}_{p4}_{pe}")
                    nc.vector.tensor_copy(bc_sb, ps_ctx[64:128, :])
                    nc.vector.tensor_tensor(dst, ps_ctx[0:64, :], bc_sb,
                                            OP.mult)

        def fc_unit(lb, last=False, use_st=False):
            # evictions split across Act (db0) and DVE (db1) so the final
            # units drain in parallel; out DMAs on Pool (SWDGE) except the
            # very last, which goes through the idle SP queue (HWDGE).
            # use_st: trailing units borrow the idle scores psum ring for db1
            # so they don't wait on cfc slots held by the final divide chains
            ost = sb.tile([128, 2, 512], BF16, tag="ostage", bufs=2,
                          name=f"ost_{lb}")
            for db in range(2):
                if use_st:
                    ps = psB.tile([128, 1024], F32, tag="st", bufs=2,
                                  name=f"fcs_{lb}_{db}")[:, 0:512]
                else:
                    ps = psB.tile([128, 512], F32, tag="cfc", bufs=4,
                                  name=f"fc_{lb}_{db}")
                for t4 in range(4):
                    nc.tensor.matmul(
                        ps, ctxT_sb[:, t4, lb * 128:lb * 128 + 128],
                        Wfc_sb[:, t4, db * 512:db * 512 + 512],
                        start=(t4 == 0), stop=(t4 == 3))
                if db == 0:
                    nc.scalar.activation(ost[:, db], ps, AF.Copy)
                else:
                    nc.vector.tensor_copy(ost[:, db], ps)
                if last:
                    eng = nc.gpsimd if db == 0 else nc.sync
                    eng.dma_start(odv[:, lb, db * 512:db * 512 + 512],
                                  ost[:, db])
            if not last:
                nc.gpsimd.dma_start(odv[:, lb, :],
                                    ost.rearrange("p db c -> p (db c)"))

        if lim >= 2:
            conv(sigq_sb, cwq_tiles, qT_sb, "q")
        if lim >= 3:
            conv(sigk_sb, cwk_tiles, kT_sb, "k")
        if lim >= 4:
            # the first vp units run on the psA ring so the psA->psB pool
            # boundary (which waits for psA quiescence) lands mid-vp, not
            # right after convk's eviction chain
            for lb in range(4 if lim >= 5 else 8):
                vp_unit(lb)

        psA.release()
        psB = tc.alloc_tile_pool(name="psB", bufs=1, space="PSUM")

        if lim >= 5:
            # the remaining vp units weave into the first two score groups to
            # cover the st-psum ring warmup; afterwards the steady pattern is
            # Sa(g) | C(g-1) | Sb(g) | fc-unit
            groups = [(qb, p4) for qb in range(2) for p4 in range(4)]
            pt0 = [[None] * 8 for _ in range(2)]
            pt1 = [[None] * 8 for _ in range(2)]
            vp_unit(4)
            scores_half(0, 0, 0, pt0)
            vp_unit(5)
            scores_half(0, 0, 1, pt0)
            vp_unit(6)
            scores_half(0, 1, 0, pt1)
            vp_unit(7)
            ctx_phase(0, 0, pt0)
            scores_half(0, 1, 1, pt1)
            pend = (0, 1, pt1)
            fc_ready = []  # lb units whose qb-half of ctxT is complete
            for qb, p4 in groups[2:]:
                pt_tiles = [[None] * 8 for _ in range(2)]
                scores_half(qb, p4, 0, pt_tiles)
                ctx_phase(*pend)
                if lim >= 6 and pend[1] == 3:
                    fc_ready.extend(range(4 * pend[0], 4 * pend[0] + 4))
                scores_half(qb, p4, 1, pt_tiles)
                if lim >= 6 and fc_ready:
                    fc_unit(fc_ready.pop(0))
                pend = (qb, p4, pt_tiles)
            if lim >= 6 and fc_ready:
                fc_unit(fc_ready.pop(0))
            ctx_phase(*pend, tail=True)
            if lim >= 6:
                if fc_ready:
                    fc_unit(fc_ready.pop(0))
                fc_ready.extend(range(4, 8))
                for lb in fc_ready:
                    fc_unit(lb, last=(lb == 7))

        psB.release()
        sb.release()
        dr.release()

    nc.finalize()
    return nc


# ----------------------------------------------------------------------------
# host-side data prep
# ----------------------------------------------------------------------------
def _host_prep(inp, flen, zero_bias):
    """Build the 8 per-core input dicts (core ci = 2*b + g)."""
    # per-parity shared tensors (g = 0, 1)
    shared = []
    for g in range(2):
        pi = np.arange(DM) ^ (512 * g)
        d = {}
        d["WQ"] = np.ascontiguousarray(inp["WQ"][:, pi]).astype(bf16)
        d["WK"] = np.ascontiguousarray(inp["WK"][:, pi]).astype(bf16)
        d["WV"] = np.ascontiguousarray(inp["WV"][:, 512 * g:512 * g + 512]).astype(bf16)
        d["Wfc"] = np.ascontiguousarray(inp["Wfc"][512 * g:512 * g + 512, :]).astype(bf16)
        if not zero_bias:
            bQ = inp["bQ"][pi].astype(np.float32)
            bK = inp["bK"][pi].astype(np.float32)
            bV = inp["bV"][512 * g:512 * g + 512].astype(np.float32)
            d["bQn"] = bQ[None, :].astype(bf16)
            d["bKn"] = bK[None, :].astype(bf16)
            d["bVn"] = bV[None, :].astype(bf16)
        for name, key in (("cwq", "conv_q"), ("cwk", "conv_k")):
            c = np.asarray(inp[key])[:, :, :flen].astype(np.float32)  # (d, o, f)
            c = np.ascontiguousarray(c.transpose(2, 0, 1))            # (f, d, o)
            c = c[:, pi, :][:, :, pi]
            # layout (8 dt, 128 p, 4 quarter, flen f, 256): column grouping so
            # each conv pass loads only its own o'-quarter of the weights
            c = c.transpose(1, 0, 2).reshape(8, 128, flen, 4, 256)
            c = np.ascontiguousarray(c.transpose(0, 1, 3, 2, 4)).reshape(8, 128, flen * 1024)
            d[name] = c.astype(bf16)
        em = np.zeros((128, 4), np.float32)
        em[:, :] = np.array([0, 0, 1, 1], np.float32) if g == 0 else \
            np.array([1, 1, 0, 0], np.float32)
        d["emask"] = em
        shared.append(d)

    maps = []
    for b in range(B):
        for g in range(2):
            sigma = np.arange(L) ^ (8 * g)
            m = dict(shared[g])
            m["Qt"] = np.ascontiguousarray(np.asarray(inp["Q"])[b][sigma, :].T).astype(bf16)
            m["Kt"] = np.ascontiguousarray(np.asarray(inp["K"])[b][sigma, :].T).astype(bf16)
            m["Vt"] = np.ascontiguousarray(np.asarray(inp["V"])[b][sigma, :].T).astype(bf16)
            maps.append(m)
    return maps


def _combine(results, inp):
    out = np.zeros((B, L, DM), np.float32)
    for b in range(B):
        for g in range(2):
            sigma = np.arange(L) ^ (8 * g)
            out[b] += np.asarray(results[2 * b + g]["out"]).astype(np.float32)[sigma, :]
        out[b] += np.asarray(inp["bfc"], dtype=np.float32)
    return out


def _get_program(flen, zero_bias=False):
    key = (flen, zero_bias)
    if key not in _CACHE:
        _CACHE[key] = _build(flen, zero_bias=zero_bias)
    return _CACHE[key]


def run_on_cores(inputs, trace=False):
    """Run the SPMD kernel; returns (full_output, BassKernelResults)."""
    from concourse.bass_utils import run_bass_kernel_spmd
    inp = {k: np.asarray(v) for k, v in inputs.items()}
    f_s = np.array(FILTER_LENGTHS, np.float32)
    flen = int(FILTER_LENGTHS[int(np.argmax(f_s * np.asarray(inp["w"], np.float32)))])
    zb = all(not np.any(np.asarray(inp[k])) for k in ("bQ", "bK", "bV"))
    nc = _get_program(flen, zero_bias=zb)
    in_maps = _host_prep(inp, flen, zb)
    res = run_bass_kernel_spmd(nc, in_maps, list(range(N_CORES)), trace=trace)
    return _combine(res.results, inp), res


def kernel(**inputs) -> np.ndarray:
    out, _ = run_on_cores(inputs, trace=False)
    return out


# revision 57
# speedup vs baseline: 1.0066x; 1.0066x over previous
# Trainium2 Bass kernel for nn_MultiHeadAttention_29154238005976 (ACAT-style conv-augmented MHA).
#
# Reference computation (B=4, L=1024, D=1024, H=16, DK=64):
#   q/k/v projections; q,k augmented by a "scrambled" depthwise-dense conv
#   (torch-style raw reshapes (b,h,l,dk)->(b, h*dk, l) scramble time/channels);
#   softmax attention per head; output projection.
#
# Sharding: 8 cores = 4 batches x 2 head-halves. All cores run an IDENTICAL
# program; per-core differences are pushed into host-side data permutations:
#   - time permutation sigma(l) = l XOR 8g applied to Q/K/V rows (g = head-half)
#   - channel permutation pi(c) = c XOR 512g applied to WQ/WK columns and to
#     conv kernel rows+columns
#   - conv "time" halves then both map to program range l2' in [0,512)
#   - boundary sig columns get a per-core 0/1 mask (true zero-padding vs
#     wrapped real data)
# Host gathers the 8 partial outputs, un-permutes rows, sums batch pairs, +bfc.
#
# Scheduling notes (cost-model driven):
#   - PE engine needs ~198us of matmul time; everything else is arranged to
#     keep PE gap-free.
#   - The hardware DGE serializes at ~630ns per DMA *instruction*, so DMAs are
#     merged into few wide access patterns (whole-cb projection staging, 2-DMA
#     sig scramble per half, per-quarter conv weights, per-lb output rows).
#   - dm-outer projection groups: the first matmul needs only one 256KB chunk.
#   - Residual transposes are taken from the projection staging tiles, so sig
#     stays 640 cols for both filter lengths and conv eviction is add-only.
#   - Attention is software-pipelined (scores of group g+1 issued before ctx
#     of group g) with the fc matmuls interleaved per qb-half.
#   - Output is written bf16 and upconverted on the host.
import numpy as np
import ml_dtypes

import concourse.bass as bass
import concourse.mybir as mybir
import concourse.tile as tile
from concourse import bacc
from concourse.masks import make_identity

bf16 = ml_dtypes.bfloat16
F32 = mybir.dt.float32
BF16 = mybir.dt.bfloat16
AF = mybir.ActivationFunctionType
OP = mybir.AluOpType

B, L, DM, H, DK = 4, 1024, 1024, 16, 64
FMAX = 4
FILTER_LENGTHS = (2, 4)
N_CORES = 8
SIGW = 640

_CACHE = {}


# ----------------------------------------------------------------------------
# program builder
# ----------------------------------------------------------------------------
def _build(flen: int, zero_bias: bool = False, upto: str = "all") -> bass.Bass:
    nc = bacc.Bacc("TRN2", target_bir_lowering=False, debug=False)

    def din(name, shape, dt):
        return nc.dram_tensor(name, list(shape), dt, kind="ExternalInput").ap()

    Qt_d = din("Qt", (DM, L), BF16)
    Kt_d = din("Kt", (DM, L), BF16)
    Vt_d = din("Vt", (DM, L), BF16)
    Wq_d = din("WQ", (DM, DM), BF16)
    Wk_d = din("WK", (DM, DM), BF16)
    Wv_d = din("WV", (DM, 512), BF16)
    Wfc_d = din("Wfc", (512, DM), BF16)
    cwq_d = din("cwq", (8, 128, flen * 1024), BF16)
    cwk_d = din("cwk", (8, 128, flen * 1024), BF16)
    if not zero_bias:
        bQn_d = din("bQn", (1, DM), BF16)
        bKn_d = din("bKn", (1, DM), BF16)
        bVn_d = din("bVn", (1, 512), BF16)
    em_d = din("emask", (128, 4), F32)
    out_d = nc.dram_tensor("out", [L, DM], BF16, kind="ExternalOutput").ap()

    CW_BUFS = 3 if flen == 2 else 2
    PT_BUFS = (12 if zero_bias else 10) if flen == 2 else 6

    with tile.TileContext(nc) as tc:
        sb = tc.alloc_tile_pool(name="sb", bufs=1)
        dr = tc.alloc_tile_pool(name="dr", bufs=1, space="DRAM")
        psA = tc.alloc_tile_pool(name="psA", bufs=1, space="PSUM")

        # ---- big loads, interleaved in first-use order ------------------
        Qt_sb = sb.tile([128, 8, L], BF16, tag="qkvt", bufs=2, name="Qt_sb")
        Wq_sb = sb.tile([128, 8, DM], BF16, name="Wq_sb")
        qtv = Qt_d.rearrange("(a p) c -> p a c", p=128)
        wqv = Wq_d.rearrange("(a p) c -> p a c", p=128)
        # first chunk split in half so the very first matmuls start sooner
        nc.sync.dma_start(Qt_sb[:, 0, 0:512], qtv[:, 0, 0:512])
        nc.sync.dma_start(Wq_sb[:, 0, 0:512], wqv[:, 0, 0:512])
        nc.sync.dma_start(Qt_sb[:, 0, 512:1024], qtv[:, 0, 512:1024])
        nc.sync.dma_start(Wq_sb[:, 0, 512:1024], wqv[:, 0, 512:1024])
        for a in range(1, 8):
            nc.sync.dma_start(Qt_sb[:, a], qtv[:, a])
            nc.sync.dma_start(Wq_sb[:, a], wqv[:, a])

        if not zero_bias:
            bQn_sb = sb.tile([1, DM], BF16, name="bQn_sb")
            nc.sync.dma_start(bQn_sb, bQn_d)
            bKn_sb = sb.tile([1, DM], BF16, name="bKn_sb")
            nc.sync.dma_start(bKn_sb, bKn_d)
            bVn_sb = sb.tile([1, 512], BF16, name="bVn_sb")
            nc.sync.dma_start(bVn_sb, bVn_d)
        else:
            bQn_sb = bKn_sb = bVn_sb = None
        em_sb = sb.tile([128, 4], F32, name="em_sb")
        nc.sync.dma_start(em_sb, em_d)

        def load_chunked2(t, dram):
            dv = dram.rearrange("(a p) c -> p a c", p=128)
            for a in range(0, 8, 2):
                nc.sync.dma_start(t[:, a:a + 2], dv[:, a:a + 2])

        Wk_sb = sb.tile([128, 8, DM], BF16, name="Wk_sb")
        load_chunked2(Wk_sb, Wk_d)
        Kt_sb = sb.tile([128, 8, L], BF16, tag="qkvt", bufs=2, name="Kt_sb")
        load_chunked2(Kt_sb, Kt_d)

        ones1_sb = sb.tile([1, 128], BF16, name="ones1_sb")
        nc.vector.memset(ones1_sb, 1.0)
        ident_sb = sb.tile([128, 128], BF16, name="ident_sb")
        make_identity(nc, ident_sb)

        qT_sb = sb.tile([128, 4, L], BF16, name="qT_sb")
        kT_sb = sb.tile([128, 4, L], BF16, name="kT_sb")
        ctxT_sb = sb.tile([128, 4, L], BF16, name="ctxT_sb")
        sigq_sb = sb.tile([128, 8, SIGW], BF16, name="sigq_sb")
        sigk_sb = sb.tile([128, 8, SIGW], BF16, name="sigk_sb")
        vpa_sb = sb.tile([128, 8, 520], BF16, name="vpa_sb")
        vpa_r = vpa_sb.rearrange("p lb (hh c) -> p lb hh c", c=65)
        nc.vector.memset(vpa_r[:, :, :, 64], 1.0)  # the denominator "ones" column

        qp_d = dr.tile([L, DM], BF16, name="qp_d")
        kp_d = dr.tile([L, DM], BF16, name="kp_d")

        # ---- phase helpers ---------------------------------------------
        def proj(Xt_sb, W_sb, bn_sb, x_d, T_sb, sig_sb, pfx):
            """x = X @ W + b staged to DRAM (for the sig scramble), with the
            own-half (cb=0) residual transposed on the fly into T_sb.
            dm-outer groups of 2 psums so the first matmul only needs one
            input chunk; evictions are pipelined one group behind; each cb
            half is staged in one SBUF tile, written with a single DMA, and
            its sig slabs are scrambled back in immediately."""
            xdv = x_d.rearrange("(lb p) c -> p lb c", p=128)
            sts = {}

            def evict(p):
                cb, lbs, pss = p
                st = sts[cb]
                for lb, ps in zip(lbs, pss):
                    nc.vector.tensor_copy(st[:, lb], ps)
                    if cb == 0:
                        trp = psA.tile([128, 512], BF16, tag="tr", bufs=2,
                                       name=f"tr_{pfx}_{lb}")
                        for ct in range(4):
                            nc.tensor.matmul(
                                trp[:, ct * 128:ct * 128 + 128],
                                st[:, lb, ct * 128:ct * 128 + 128],
                                ident_sb, is_transpose=True,
                                start=(ct == 0), stop=(ct == 3))
                        # on DVE: Act is busy issuing the sig DMA chain and
                        # GPSIMD cannot read PSUM
                        nc.vector.tensor_copy(
                            T_sb[:, :, lb * 128:lb * 128 + 128],
                            trp.rearrange("p (ct r) -> p ct r", ct=4))
                if lbs[-1] == 7:
                    nc.scalar.dma_start(xdv[:, :, cb * 512:cb * 512 + 512], st)
                    scramble_half(x_d, sig_sb, cb)

            pend = None
            for cb in range(2):
                sts[cb] = sb.tile([128, 8, 512], BF16, tag="stage", bufs=2,
                                  name=f"st_{pfx}_{cb}")
                for g in range(4):
                    lbs = [2 * g, 2 * g + 1]
                    pss = []
                    for lb in lbs:
                        ps = psA.tile([128, 512], F32, tag="A", bufs=6,
                                      name=f"ps_{pfx}_{cb}_{lb}")
                        if not zero_bias:
                            nc.tensor.matmul(ps, ones1_sb[0:1, :],
                                             bn_sb[0:1, cb * 512:cb * 512 + 512],
                                             start=True, stop=False)
                        pss.append(ps)
                    for dm in range(8):
                        for i, lb in enumerate(lbs):
                            nc.tensor.matmul(
                                pss[i],
                                Xt_sb[:, dm, lb * 128:lb * 128 + 128],
                                W_sb[:, dm, cb * 512:cb * 512 + 512],
                                start=(zero_bias and dm == 0), stop=(dm == 7))
                    if pend is not None:
                        evict(pend)
                    pend = (cb, lbs, pss)
            evict(pend)

        def scramble_half(x_d, sig_sb, cb):
            """sig[64*hib + il, dt, cols] <- qp[16*il + jh, 128*dt + 64*hib + jl]
            for the dt slabs derived from channel half cb (dt in [4cb, 4cb+4)).
            sig cols [0,64) = j in [960,1024); cols [64,640) = j in [0,576)."""
            # on the Pool queue: keeps the DVE/Act queues free of the
            # sig-load dependency chain (head-of-line blocking)
            xr = x_d.rearrange("(il jh) (dt hib jl) -> hib il dt jh jl",
                               jh=16, hib=2, jl=64)
            dts = slice(4 * cb, 4 * cb + 4)
            for hib in range(2):
                for dt in range(4 * cb, 4 * cb + 4):
                    dst = sig_sb[64 * hib:64 * hib + 64, dt, 64:64 + 64 * 9]
                    nc.scalar.dma_start(
                        dst.rearrange("p (jh jl) -> p jh jl", jl=64),
                        xr[hib, :, dt, 0:9, :])
                nc.scalar.dma_start(sig_sb[64 * hib:64 * hib + 64, dts, 0:64],
                                    xr[hib, :, dts, 15, :])
            # mask the wrap/pad boundary columns (j' = -2,-1[,512,513])
            nc.gpsimd.tensor_tensor(
                sig_sb[:, dts, 62:64], sig_sb[:, dts, 62:64],
                em_sb[:, None, 0:2].to_broadcast((128, 4, 2)), OP.mult)
            if flen == 4:
                nc.gpsimd.tensor_tensor(
                    sig_sb[:, dts, 576:578], sig_sb[:, dts, 576:578],
                    em_sb[:, None, 2:4].to_broadcast((128, 4, 2)), OP.mult)

        def load_cw(cw_d, pfx):
            """One DMA per o'-quarter of the conv weights."""
            cwv = cw_d.rearrange("dt p x -> p dt x")
            tiles = []
            for quarter in range(4):
                t = sb.tile([128, 8, flen * 256], BF16, tag="cw", bufs=CW_BUFS,
                            name=f"cw_{pfx}_{quarter}")
                nc.sync.dma_start(
                    t, cwv[:, :, quarter * flen * 256:(quarter + 1) * flen * 256])
                tiles.append(t)
            return tiles

        def conv(sig_sb, cw_tiles, T_sb, pfx):
            """T[c,l] += conv output, scrambled back into head-transposed tiles.
            conv psum tile s: partition o' = 128 s + 64 ph + k, free l2' = 64 hl + m."""
            Tr = T_sb.rearrange("p q (m r) -> p q m r", r=16)
            for quarter in range(4):
                pss = [psA.tile([128, 512], F32, tag="A", bufs=6,
                                name=f"cps_{pfx}_{quarter}_{si}") for si in range(2)]
                cwt = cw_tiles[quarter]
                for dt in range(8):
                    for si in range(2):
                        for f in range(flen):
                            nc.tensor.matmul(
                                pss[si],
                                cwt[:, dt, f * 256 + si * 128:f * 256 + si * 128 + 128],
                                sig_sb[:, dt, 62 + f:62 + f + 512],
                                start=(dt == 0 and f == 0),
                                stop=(dt == 7 and f == flen - 1))
                for si in range(2):
                    sblk = 2 * quarter + si
                    ps = pss[si].rearrange("p (q h m) -> p q h m", q=4, h=2)
                    for ph in range(2):
                        for pe in range(2):
                            dst = Tr[64 * pe:64 * pe + 64, :, :, 2 * sblk + ph]
                            nc.vector.tensor_tensor(
                                dst, ps[64 * ph:64 * ph + 64, :, pe, :], dst,
                                OP.add)

        # ---- phase sequence --------------------------------------------
        PHASES = ["q", "k", "convq", "convk", "vp", "attn", "all"]
        lim = PHASES.index(upto)

        proj(Qt_sb, Wq_sb, bQn_sb, qp_d, qT_sb, sigq_sb, "q")

        if lim >= 1:
            proj(Kt_sb, Wk_sb, bKn_sb, kp_d, kT_sb, sigk_sb, "k")

        # conv weight prefetch + late big loads. Order matters: the cw ring
        # (bufs=3) stalls the SP queue as conv consumes tiles, which keeps
        # Vt/Wv/Wfc transfers from competing with the sig scramble reads.
        cwq_tiles = load_cw(cwq_d, "q")
        cwk_tiles = load_cw(cwk_d, "k")
        Vt_sb = sb.tile([128, 8, L], BF16, tag="qkvt", bufs=2, name="Vt_sb")
        load_chunked2(Vt_sb, Vt_d)
        Wv_sb = sb.tile([128, 8, 512], BF16, name="Wv_sb")
        nc.sync.dma_start(Wv_sb, Wv_d.rearrange("(a p) c -> p a c", p=128))
        Wfc_sb = sb.tile([128, 4, DM], BF16, name="Wfc_sb")
        nc.sync.dma_start(Wfc_sb, Wfc_d.rearrange("(t p) c -> p t c", p=128))

        # ---- v-projection + attention + fc (software-pipelined) ---------
        odv = out_d.rearrange("(lb p) c -> p lb c", p=128)
        psB = None

        def vp_unit(lb):
            if psB is None:
                ps = psA.tile([128, 512], F32, tag="A", bufs=6,
                              name=f"psv_{lb}")
            else:
                ps = psB.tile([128, 512], F32, tag="cfc", bufs=4,
                              name=f"psv_{lb}")
            if not zero_bias:
                nc.tensor.matmul(ps, ones1_sb[0:1, :], bVn_sb[0:1, :],
                                 start=True, stop=False)
            for dm in range(8):
                nc.tensor.matmul(ps, Vt_sb[:, dm, lb * 128:lb * 128 + 128],
                                 Wv_sb[:, dm, :],
                                 start=(zero_bias and dm == 0), stop=(dm == 7))
            nc.vector.tensor_copy(
                vpa_r[:, lb, :, 0:64],
                ps.rearrange("p (hh c) -> p hh c", hh=8))

        def scores_half(qb, p4, half, pt_tiles):
            """QK^T + exp for two kt2 blocks of one (qb, p4) group."""
            for kt2 in (0, 1) if half == 0 else (2, 3):
                for pe in range(2):
                    ps_st = psB.tile([128, 1024], F32, tag="st", bufs=2,
                                     name=f"st_{qb}_{p4}_{kt2}_{pe}")
                    for h in range(2):
                        kt = 2 * kt2 + h
                        nc.tensor.matmul(
                            ps_st[:, 512 * h:512 * h + 512],
                            kT_sb[64 * pe:64 * pe + 64, p4, kt * 128:kt * 128 + 128],
                            qT_sb[64 * pe:64 * pe + 64, p4, qb * 512:qb * 512 + 512],
                            start=True, stop=True, tile_position=(64 * pe, 0))
                    pt = sb.tile([128, 1024], BF16, tag="pt", bufs=PT_BUFS,
                                 name=f"pt_{qb}_{p4}_{kt2}_{pe}")
                    nc.scalar.activation(pt, ps_st, AF.Exp, scale=0.125)
                    pt_tiles[pe][2 * kt2] = pt[:, 0:512]
                    pt_tiles[pe][2 * kt2 + 1] = pt[:, 512:1024]

        def ctx_phase(qb, p4, pt_tiles, tail=False):
            for pe in range(2):
                hl = 2 * p4 + pe
                # ctx accumulates in rows [0,65); the reciprocal broadcast
                # reuses rows [64,128) of the same bank after the denominator
                # row has been consumed
                ps_ctx = psB.tile([128, 512], F32, tag="cfc", bufs=4,
                                  name=f"ctx_{qb}_{p4}_{pe}")
                for kt in range(8):
                    nc.tensor.matmul(
                        ps_ctx[0:65, :], vpa_sb[:, kt, 65 * hl:65 * hl + 65],
                        pt_tiles[pe][kt], start=(kt == 0), stop=(kt == 7))
                rcb = sb.tile([1, 512], BF16, tag="recipb", bufs=2,
                              name=f"rcb_{qb}_{p4}_{pe}")
                if tail:
                    # shortcut for the final groups (where the divide chain is
                    # exposed): stage the unnormalized ctx rows on the
                    # now-idle Act engine in parallel with the reciprocal
                    un = sb.tile([64, 512], F32, tag="bcs", bufs=2,
                                 name=f"un_{qb}_{p4}_{pe}")
                    nc.scalar.activation(un, ps_ctx[0:64, :], AF.Copy)
                with nc.allow_low_precision(reason="softmax denominators are "
                                            "O(100); bf16 reciprocal is ample"):
                    nc.vector.reciprocal(rcb, ps_ctx[64:65, :])
                nc.tensor.matmul(ps_ctx[64:128, :], ones1_sb[0:1, 0:64], rcb,
                                 start=True, stop=True)
                dst = ctxT_sb[64 * pe:64 * pe + 64, p4, qb * 512:qb * 512 + 512]
                if tail:
                    nc.vector.tensor_tensor(dst, un, ps_ctx[64:128, :], OP.mult)
                else:
                    bc_sb = sb.tile([64, 512], F32, tag="bcs", bufs=2,
                                    name=f"bcs_{qb}_{p4}_{pe}")
                    nc.vector.tensor_copy(bc_sb, ps_ctx[64:128, :])
                    nc.vector.tensor_tensor(dst, ps_ctx[0:64, :], bc_sb,
                                            OP.mult)

        def fc_unit(lb, last=False, use_st=False):
            # evictions split across Act (db0) and DVE (db1) so the final
            # units drain in parallel; out DMAs on Pool (SWDGE) except the
            # very last, which goes through the idle SP queue (HWDGE).
            # use_st: trailing units borrow the idle scores psum ring for db1
            # so they don't wait on cfc slots held by the final divide chains
            ost = sb.tile([128, 2, 512], BF16, tag="ostage", bufs=2,
                          name=f"ost_{lb}")
            for db in range(2):
                if use_st:
                    ps = psB.tile([128, 1024], F32, tag="st", bufs=2,
                                  name=f"fcs_{lb}_{db}")[:, 0:512]
                else:
                    ps = psB.tile([128, 512], F32, tag="cfc", bufs=4,
                                  name=f"fc_{lb}_{db}")
                for t4 in range(4):
                    nc.tensor.matmul(
                        ps, ctxT_sb[:, t4, lb * 128:lb * 128 + 128],
                        Wfc_sb[:, t4, db * 512:db * 512 + 512],
                        start=(t4 == 0), stop=(t4 == 3))
                if db == 0:
                    nc.scalar.activation(ost[:, db], ps, AF.Copy)
                else:
                    nc.vector.tensor_copy(ost[:, db], ps)
                if last:
                    eng = nc.gpsimd if db == 0 else nc.sync
                    eng.dma_start(odv[:, lb, db * 512:db * 512 + 512],
                                  ost[:, db])
            if not last:
                nc.gpsimd.dma_start(odv[:, lb, :],
                                    ost.rearrange("p db c -> p (db c)"))

        if lim >= 2:
            conv(sigq_sb, cwq_tiles, qT_sb, "q")
        if lim >= 3:
            conv(sigk_sb, cwk_tiles, kT_sb, "k")
        if lim >= 4:
            # the first vp units run on the psA ring so the psA->psB pool
            # boundary (which waits for psA quiescence) lands mid-vp, not
            # right after convk's eviction chain
            for lb in range(4 if lim >= 5 else 8):
                vp_unit(lb)

        psA.release()
        psB = tc.alloc_tile_pool(name="psB", bufs=1, space="PSUM")

        if lim >= 5:
            # the remaining vp units weave into the first two score groups to
            # cover the st-psum ring warmup; afterwards the steady pattern is
            # Sa(g) | C(g-1) | Sb(g) | fc-unit
            groups = [(qb, p4) for qb in range(2) for p4 in range(4)]
            pt0 = [[None] * 8 for _ in range(2)]
            pt1 = [[None] * 8 for _ in range(2)]
            vp_unit(4)
            scores_half(0, 0, 0, pt0)
            vp_unit(5)
            scores_half(0, 0, 1, pt0)
            vp_unit(6)
            scores_half(0, 1, 0, pt1)
            vp_unit(7)
            ctx_phase(0, 0, pt0)
            scores_half(0, 1, 1, pt1)
            pend = (0, 1, pt1)
            fc_ready = []  # lb units whose qb-half of ctxT is complete
            for qb, p4 in groups[2:]:
                pt_tiles = [[None] * 8 for _ in range(2)]
                scores_half(qb, p4, 0, pt_tiles)
                ctx_phase(*pend)
                if lim >= 6 and pend[1] == 3:
                    fc_ready.extend(range(4 * pend[0], 4 * pend[0] + 4))
                scores_half(qb, p4, 1, pt_tiles)
                if lim >= 6 and fc_ready:
                    fc_unit(fc_ready.pop(0))
                pend = (qb, p4, pt_tiles)
            if lim >= 6 and fc_ready:
                fc_unit(fc_ready.pop(0))
            ctx_phase(*pend, tail=True)
            if lim >= 6:
                if fc_ready:
                    fc_unit(fc_ready.pop(0))
                fc_ready.extend(range(4, 8))
                for lb in fc_ready:
                    fc_unit(lb, last=(lb == 7))

        psB.release()
        sb.release()
        dr.release()

    nc.finalize()
    return nc


# ----------------------------------------------------------------------------
# host-side data prep
# ----------------------------------------------------------------------------
def _host_prep(inp, flen, zero_bias):
    """Build the 8 per-core input dicts (core ci = 2*b + g)."""
    # per-parity shared tensors (g = 0, 1)
    shared = []
    for g in range(2):
        pi = np.arange(DM) ^ (512 * g)
        d = {}
        d["WQ"] = np.ascontiguousarray(inp["WQ"][:, pi]).astype(bf16)
        d["WK"] = np.ascontiguousarray(inp["WK"][:, pi]).astype(bf16)
        d["WV"] = np.ascontiguousarray(inp["WV"][:, 512 * g:512 * g + 512]).astype(bf16)
        d["Wfc"] = np.ascontiguousarray(inp["Wfc"][512 * g:512 * g + 512, :]).astype(bf16)
        if not zero_bias:
            bQ = inp["bQ"][pi].astype(np.float32)
            bK = inp["bK"][pi].astype(np.float32)
            bV = inp["bV"][512 * g:512 * g + 512].astype(np.float32)
            d["bQn"] = bQ[None, :].astype(bf16)
            d["bKn"] = bK[None, :].astype(bf16)
            d["bVn"] = bV[None, :].astype(bf16)
        for name, key in (("cwq", "conv_q"), ("cwk", "conv_k")):
            c = np.asarray(inp[key])[:, :, :flen].astype(np.float32)  # (d, o, f)
            c = np.ascontiguousarray(c.transpose(2, 0, 1))            # (f, d, o)
            c = c[:, pi, :][:, :, pi]
            # layout (8 dt, 128 p, 4 quarter, flen f, 256): column grouping so
            # each conv pass loads only its own o'-quarter of the weights
            c = c.transpose(1, 0, 2).reshape(8, 128, flen, 4, 256)
            c = np.ascontiguousarray(c.transpose(0, 1, 3, 2, 4)).reshape(8, 128, flen * 1024)
            d[name] = c.astype(bf16)
        em = np.zeros((128, 4), np.float32)
        em[:, :] = np.array([0, 0, 1, 1], np.float32) if g == 0 else \
            np.array([1, 1, 0, 0], np.float32)
        d["emask"] = em
        shared.append(d)

    maps = []
    for b in range(B):
        for g in range(2):
            sigma = np.arange(L) ^ (8 * g)
            m = dict(shared[g])
            m["Qt"] = np.ascontiguousarray(np.asarray(inp["Q"])[b][sigma, :].T).astype(bf16)
            m["Kt"] = np.ascontiguousarray(np.asarray(inp["K"])[b][sigma, :].T).astype(bf16)
            m["Vt"] = np.ascontiguousarray(np.asarray(inp["V"])[b][sigma, :].T).astype(bf16)
            maps.append(m)
    return maps


def _combine(results, inp):
    out = np.zeros((B, L, DM), np.float32)
    for b in range(B):
        for g in range(2):
            sigma = np.arange(L) ^ (8 * g)
            out[b] += np.asarray(results[2 * b + g]["out"]).astype(np.float32)[sigma, :]
        out[b] += np.asarray(inp["bfc"], dtype=np.float32)
    return out


def _get_program(flen, zero_bias=False):
    key = (flen, zero_bias)
    if key not in _CACHE:
        _CACHE[key] = _build(flen, zero_bias=zero_bias)
    return _CACHE[key]


def run_on_cores(inputs, trace=False):
    """Run the SPMD kernel; returns (full_output, BassKernelResults)."""
    from concourse.bass_utils import run_bass_kernel_spmd
    inp = {k: np.asarray(v) for k, v in inputs.items()}
    f_s = np.array(FILTER_LENGTHS, np.float32)
    flen = int(FILTER_LENGTHS[int(np.argmax(f_s * np.asarray(inp["w"], np.float32)))])
    zb = all(not np.any(np.asarray(inp[k])) for k in ("bQ", "bK", "bV"))
    nc = _get_program(flen, zero_bias=zb)
    in_maps = _host_prep(inp, flen, zb)
    res = run_bass_kernel_spmd(nc, in_maps, list(range(N_CORES)), trace=trace)
    return _combine(res.results, inp), res


def kernel(**inputs) -> np.ndarray:
    out, _ = run_on_cores(inputs, trace=False)
    return out


# revision 60
# speedup vs baseline: 1.0131x; 1.0064x over previous
# Trainium2 Bass kernel for nn_MultiHeadAttention_29154238005976 (ACAT-style conv-augmented MHA).
#
# Reference computation (B=4, L=1024, D=1024, H=16, DK=64):
#   q/k/v projections; q,k augmented by a "scrambled" depthwise-dense conv
#   (torch-style raw reshapes (b,h,l,dk)->(b, h*dk, l) scramble time/channels);
#   softmax attention per head; output projection.
#
# Sharding: 8 cores = 4 batches x 2 head-halves. All cores run an IDENTICAL
# program; per-core differences are pushed into host-side data permutations:
#   - time permutation sigma(l) = l XOR 8g applied to Q/K/V rows (g = head-half)
#   - channel permutation pi(c) = c XOR 512g applied to WQ/WK columns and to
#     conv kernel rows+columns
#   - conv "time" halves then both map to program range l2' in [0,512)
#   - boundary sig columns get a per-core 0/1 mask (true zero-padding vs
#     wrapped real data)
# Host gathers the 8 partial outputs, un-permutes rows, sums batch pairs, +bfc.
#
# Scheduling notes (cost-model driven):
#   - PE engine needs ~198us of matmul time; everything else is arranged to
#     keep PE gap-free.
#   - The hardware DGE serializes at ~630ns per DMA *instruction*, so DMAs are
#     merged into few wide access patterns (whole-cb projection staging, 2-DMA
#     sig scramble per half, per-quarter conv weights, per-lb output rows).
#   - dm-outer projection groups: the first matmul needs only one 256KB chunk.
#   - Residual transposes are taken from the projection staging tiles, so sig
#     stays 640 cols for both filter lengths and conv eviction is add-only.
#   - Attention is software-pipelined (scores of group g+1 issued before ctx
#     of group g) with the fc matmuls interleaved per qb-half.
#   - Output is written bf16 and upconverted on the host.
import numpy as np
import ml_dtypes

import concourse.bass as bass
import concourse.mybir as mybir
import concourse.tile as tile
from concourse import bacc
from concourse.masks import make_identity

bf16 = ml_dtypes.bfloat16
F32 = mybir.dt.float32
BF16 = mybir.dt.bfloat16
AF = mybir.ActivationFunctionType
OP = mybir.AluOpType

B, L, DM, H, DK = 4, 1024, 1024, 16, 64
FMAX = 4
FILTER_LENGTHS = (2, 4)
N_CORES = 8
SIGW = 640

_CACHE = {}


# ----------------------------------------------------------------------------
# program builder
# ----------------------------------------------------------------------------
def _build(flen: int, zero_bias: bool = False, upto: str = "all") -> bass.Bass:
    nc = bacc.Bacc("TRN2", target_bir_lowering=False, debug=False)

    def din(name, shape, dt):
        return nc.dram_tensor(name, list(shape), dt, kind="ExternalInput").ap()

    Qt_d = din("Qt", (DM, L), BF16)
    Kt_d = din("Kt", (DM, L), BF16)
    Vt_d = din("Vt", (DM, L), BF16)
    Wq_d = din("WQ", (DM, DM), BF16)
    Wk_d = din("WK", (DM, DM), BF16)
    Wv_d = din("WV", (DM, 512), BF16)
    Wfc_d = din("Wfc", (512, DM), BF16)
    cwq_d = din("cwq", (8, 128, flen * 1024), BF16)
    cwk_d = din("cwk", (8, 128, flen * 1024), BF16)
    if not zero_bias:
        bQn_d = din("bQn", (1, DM), BF16)
        bKn_d = din("bKn", (1, DM), BF16)
        bVn_d = din("bVn", (1, 512), BF16)
    em_d = din("emask", (128, 4), F32)
    out_d = nc.dram_tensor("out", [L, DM], BF16, kind="ExternalOutput").ap()

    CW_BUFS = 3 if flen == 2 else 2
    PT_BUFS = (12 if zero_bias else 10) if flen == 2 else 6

    with tile.TileContext(nc) as tc:
        sb = tc.alloc_tile_pool(name="sb", bufs=1)
        dr = tc.alloc_tile_pool(name="dr", bufs=1, space="DRAM")
        psA = tc.alloc_tile_pool(name="psA", bufs=1, space="PSUM")

        # ---- big loads, interleaved in first-use order ------------------
        Qt_sb = sb.tile([128, 8, L], BF16, tag="qkvt", bufs=2, name="Qt_sb")
        Wq_sb = sb.tile([128, 8, DM], BF16, name="Wq_sb")
        qtv = Qt_d.rearrange("(a p) c -> p a c", p=128)
        wqv = Wq_d.rearrange("(a p) c -> p a c", p=128)
        # first chunk split in half so the very first matmuls start sooner
        nc.sync.dma_start(Qt_sb[:, 0, 0:512], qtv[:, 0, 0:512])
        nc.sync.dma_start(Wq_sb[:, 0, 0:512], wqv[:, 0, 0:512])
        nc.sync.dma_start(Qt_sb[:, 0, 512:1024], qtv[:, 0, 512:1024])
        nc.sync.dma_start(Wq_sb[:, 0, 512:1024], wqv[:, 0, 512:1024])
        for a in range(1, 8):
            nc.sync.dma_start(Qt_sb[:, a], qtv[:, a])
            nc.sync.dma_start(Wq_sb[:, a], wqv[:, a])

        if not zero_bias:
            bQn_sb = sb.tile([1, DM], BF16, name="bQn_sb")
            nc.sync.dma_start(bQn_sb, bQn_d)
            bKn_sb = sb.tile([1, DM], BF16, name="bKn_sb")
            nc.sync.dma_start(bKn_sb, bKn_d)
            bVn_sb = sb.tile([1, 512], BF16, name="bVn_sb")
            nc.sync.dma_start(bVn_sb, bVn_d)
        else:
            bQn_sb = bKn_sb = bVn_sb = None
        em_sb = sb.tile([128, 4], F32, name="em_sb")
        nc.sync.dma_start(em_sb, em_d)

        def load_chunked2(t, dram):
            dv = dram.rearrange("(a p) c -> p a c", p=128)
            for a in range(0, 8, 2):
                nc.sync.dma_start(t[:, a:a + 2], dv[:, a:a + 2])

        Wk_sb = sb.tile([128, 8, DM], BF16, name="Wk_sb")
        load_chunked2(Wk_sb, Wk_d)
        Kt_sb = sb.tile([128, 8, L], BF16, tag="qkvt", bufs=2, name="Kt_sb")
        load_chunked2(Kt_sb, Kt_d)

        ones1_sb = sb.tile([1, 128], BF16, name="ones1_sb")
        nc.vector.memset(ones1_sb, 1.0)
        ident_sb = sb.tile([128, 128], BF16, name="ident_sb")
        make_identity(nc, ident_sb)

        qT_sb = sb.tile([128, 4, L], BF16, name="qT_sb")
        kT_sb = sb.tile([128, 4, L], BF16, name="kT_sb")
        ctxT_sb = sb.tile([128, 4, L], BF16, name="ctxT_sb")
        sigq_sb = sb.tile([128, 8, SIGW], BF16, name="sigq_sb")
        sigk_sb = sb.tile([128, 8, SIGW], BF16, name="sigk_sb")
        vpa_sb = sb.tile([128, 8, 520], BF16, name="vpa_sb")
        vpa_r = vpa_sb.rearrange("p lb (hh c) -> p lb hh c", c=65)
        nc.vector.memset(vpa_r[:, :, :, 64], 1.0)  # the denominator "ones" column

        qp_d = dr.tile([L, DM], BF16, name="qp_d")
        kp_d = dr.tile([L, DM], BF16, name="kp_d")

        # ---- phase helpers ---------------------------------------------
        def proj(Xt_sb, W_sb, bn_sb, x_d, T_sb, sig_sb, pfx):
            """x = X @ W + b staged to DRAM (for the sig scramble), with the
            own-half (cb=0) residual transposed on the fly into T_sb.
            dm-outer groups of 2 psums so the first matmul only needs one
            input chunk; evictions are pipelined one group behind; each cb
            half is staged in one SBUF tile, written with a single DMA, and
            its sig slabs are scrambled back in immediately."""
            xdv = x_d.rearrange("(lb p) c -> p lb c", p=128)
            sts = {}

            def evict(p):
                cb, lbs, pss = p
                st = sts[cb]
                # both lbs' transposes pack into ONE bf16 psum tile (one 2KB
                # bank), leaving 7 banks for the projection ring; one merged
                # residual copy (on DVE: Act is busy issuing the sig DMA
                # chain and GPSIMD cannot read PSUM)
                trp = None
                if cb == 0:
                    trp = psA.tile([128, 1024], BF16, tag="tr", bufs=1,
                                   name=f"tr_{pfx}_{lbs[0]}")
                for i, (lb, ps) in enumerate(zip(lbs, pss)):
                    nc.vector.tensor_copy(st[:, lb], ps)
                    if cb == 0:
                        for ct in range(4):
                            nc.tensor.matmul(
                                trp[:, i * 512 + ct * 128:i * 512 + ct * 128 + 128],
                                st[:, lb, ct * 128:ct * 128 + 128],
                                ident_sb, is_transpose=True,
                                start=(i == 0 and ct == 0),
                                stop=(i == 1 and ct == 3))
                if cb == 0:
                    dst = T_sb[:, :, lbs[0] * 128:lbs[0] * 128 + 256]
                    nc.vector.tensor_copy(
                        dst.rearrange("p q (lb r) -> p q lb r", r=128),
                        trp.rearrange("p (lb ct r) -> p ct lb r", lb=2, r=128))
                if lbs[-1] == 7:
                    nc.scalar.dma_start(xdv[:, :, cb * 512:cb * 512 + 512], st)
                    scramble_half(x_d, sig_sb, cb)

            pend = None
            for cb in range(2):
                sts[cb] = sb.tile([128, 8, 512], BF16, tag="stage", bufs=2,
                                  name=f"st_{pfx}_{cb}")
                for g in range(4):
                    lbs = [2 * g, 2 * g + 1]
                    pss = []
                    for lb in lbs:
                        ps = psA.tile([128, 512], F32, tag="A", bufs=7,
                                      name=f"ps_{pfx}_{cb}_{lb}")
                        if not zero_bias:
                            nc.tensor.matmul(ps, ones1_sb[0:1, :],
                                             bn_sb[0:1, cb * 512:cb * 512 + 512],
                                             start=True, stop=False)
                        pss.append(ps)
                    for dm in range(8):
                        for i, lb in enumerate(lbs):
                            nc.tensor.matmul(
                                pss[i],
                                Xt_sb[:, dm, lb * 128:lb * 128 + 128],
                                W_sb[:, dm, cb * 512:cb * 512 + 512],
                                start=(zero_bias and dm == 0), stop=(dm == 7))
                    if pend is not None:
                        evict(pend)
                    pend = (cb, lbs, pss)
            evict(pend)

        def scramble_half(x_d, sig_sb, cb):
            """sig[64*hib + il, dt, cols] <- qp[16*il + jh, 128*dt + 64*hib + jl]
            for the dt slabs derived from channel half cb (dt in [4cb, 4cb+4)).
            sig cols [0,64) = j in [960,1024); cols [64,640) = j in [0,576)."""
            # on the Pool queue: keeps the DVE/Act queues free of the
            # sig-load dependency chain (head-of-line blocking)
            xr = x_d.rearrange("(il jh) (dt hib jl) -> hib il dt jh jl",
                               jh=16, hib=2, jl=64)
            dts = slice(4 * cb, 4 * cb + 4)
            for hib in range(2):
                for dt in range(4 * cb, 4 * cb + 4):
                    dst = sig_sb[64 * hib:64 * hib + 64, dt, 64:64 + 64 * 9]
                    nc.scalar.dma_start(
                        dst.rearrange("p (jh jl) -> p jh jl", jl=64),
                        xr[hib, :, dt, 0:9, :])
                nc.scalar.dma_start(sig_sb[64 * hib:64 * hib + 64, dts, 0:64],
                                    xr[hib, :, dts, 15, :])
            # mask the wrap/pad boundary columns (j' = -2,-1[,512,513])
            nc.gpsimd.tensor_tensor(
                sig_sb[:, dts, 62:64], sig_sb[:, dts, 62:64],
                em_sb[:, None, 0:2].to_broadcast((128, 4, 2)), OP.mult)
            if flen == 4:
                nc.gpsimd.tensor_tensor(
                    sig_sb[:, dts, 576:578], sig_sb[:, dts, 576:578],
                    em_sb[:, None, 2:4].to_broadcast((128, 4, 2)), OP.mult)

        def load_cw(cw_d, pfx):
            """One DMA per o'-quarter of the conv weights."""
            cwv = cw_d.rearrange("dt p x -> p dt x")
            tiles = []
            for quarter in range(4):
                t = sb.tile([128, 8, flen * 256], BF16, tag="cw", bufs=CW_BUFS,
                            name=f"cw_{pfx}_{quarter}")
                nc.sync.dma_start(
                    t, cwv[:, :, quarter * flen * 256:(quarter + 1) * flen * 256])
                tiles.append(t)
            return tiles

        def conv(sig_sb, cw_tiles, T_sb, pfx):
            """T[c,l] += conv output, scrambled back into head-transposed tiles.
            conv psum tile s: partition o' = 128 s + 64 ph + k, free l2' = 64 hl + m."""
            Tr = T_sb.rearrange("p q (m r) -> p q m r", r=16)
            for quarter in range(4):
                pss = [psA.tile([128, 512], F32, tag="A", bufs=7,
                                name=f"cps_{pfx}_{quarter}_{si}") for si in range(2)]
                cwt = cw_tiles[quarter]
                for dt in range(8):
                    for si in range(2):
                        for f in range(flen):
                            nc.tensor.matmul(
                                pss[si],
                                cwt[:, dt, f * 256 + si * 128:f * 256 + si * 128 + 128],
                                sig_sb[:, dt, 62 + f:62 + f + 512],
                                start=(dt == 0 and f == 0),
                                stop=(dt == 7 and f == flen - 1))
                for si in range(2):
                    sblk = 2 * quarter + si
                    ps = pss[si].rearrange("p (q h m) -> p q h m", q=4, h=2)
                    for ph in range(2):
                        for pe in range(2):
                            dst = Tr[64 * pe:64 * pe + 64, :, :, 2 * sblk + ph]
                            nc.vector.tensor_tensor(
                                dst, ps[64 * ph:64 * ph + 64, :, pe, :], dst,
                                OP.add)

        # ---- phase sequence --------------------------------------------
        PHASES = ["q", "k", "convq", "convk", "vp", "attn", "all"]
        lim = PHASES.index(upto)

        proj(Qt_sb, Wq_sb, bQn_sb, qp_d, qT_sb, sigq_sb, "q")

        if lim >= 1:
            proj(Kt_sb, Wk_sb, bKn_sb, kp_d, kT_sb, sigk_sb, "k")

        # conv weight prefetch + late big loads. Order matters: the cw ring
        # (bufs=3) stalls the SP queue as conv consumes tiles, which keeps
        # Vt/Wv/Wfc transfers from competing with the sig scramble reads.
        cwq_tiles = load_cw(cwq_d, "q")
        cwk_tiles = load_cw(cwk_d, "k")
        Vt_sb = sb.tile([128, 8, L], BF16, tag="qkvt", bufs=2, name="Vt_sb")
        load_chunked2(Vt_sb, Vt_d)
        Wv_sb = sb.tile([128, 8, 512], BF16, name="Wv_sb")
        nc.sync.dma_start(Wv_sb, Wv_d.rearrange("(a p) c -> p a c", p=128))
        Wfc_sb = sb.tile([128, 4, DM], BF16, name="Wfc_sb")
        nc.sync.dma_start(Wfc_sb, Wfc_d.rearrange("(t p) c -> p t c", p=128))

        # ---- v-projection + attention + fc (software-pipelined) ---------
        odv = out_d.rearrange("(lb p) c -> p lb c", p=128)
        psB = None

        def vp_unit(lb):
            if psB is None:
                ps = psA.tile([128, 512], F32, tag="A", bufs=7,
                              name=f"psv_{lb}")
            else:
                ps = psB.tile([128, 512], F32, tag="cfc", bufs=4,
                              name=f"psv_{lb}")
            if not zero_bias:
                nc.tensor.matmul(ps, ones1_sb[0:1, :], bVn_sb[0:1, :],
                                 start=True, stop=False)
            for dm in range(8):
                nc.tensor.matmul(ps, Vt_sb[:, dm, lb * 128:lb * 128 + 128],
                                 Wv_sb[:, dm, :],
                                 start=(zero_bias and dm == 0), stop=(dm == 7))
            nc.vector.tensor_copy(
                vpa_r[:, lb, :, 0:64],
                ps.rearrange("p (hh c) -> p hh c", hh=8))

        def scores_half(qb, p4, half, pt_tiles):
            """QK^T + exp for two kt2 blocks of one (qb, p4) group."""
            for kt2 in (0, 1) if half == 0 else (2, 3):
                for pe in range(2):
                    ps_st = psB.tile([128, 1024], F32, tag="st", bufs=2,
                                     name=f"st_{qb}_{p4}_{kt2}_{pe}")
                    for h in range(2):
                        kt = 2 * kt2 + h
                        nc.tensor.matmul(
                            ps_st[:, 512 * h:512 * h + 512],
                            kT_sb[64 * pe:64 * pe + 64, p4, kt * 128:kt * 128 + 128],
                            qT_sb[64 * pe:64 * pe + 64, p4, qb * 512:qb * 512 + 512],
                            start=True, stop=True, tile_position=(64 * pe, 0))
                    pt = sb.tile([128, 1024], BF16, tag="pt", bufs=PT_BUFS,
                                 name=f"pt_{qb}_{p4}_{kt2}_{pe}")
                    nc.scalar.activation(pt, ps_st, AF.Exp, scale=0.125)
                    pt_tiles[pe][2 * kt2] = pt[:, 0:512]
                    pt_tiles[pe][2 * kt2 + 1] = pt[:, 512:1024]

        def ctx_phase(qb, p4, pt_tiles, tail=False):
            for pe in range(2):
                hl = 2 * p4 + pe
                # ctx accumulates in rows [0,65); the reciprocal broadcast
                # reuses rows [64,128) of the same bank after the denominator
                # row has been consumed
                ps_ctx = psB.tile([128, 512], F32, tag="cfc", bufs=4,
                                  name=f"ctx_{qb}_{p4}_{pe}")
                for kt in range(8):
                    nc.tensor.matmul(
                        ps_ctx[0:65, :], vpa_sb[:, kt, 65 * hl:65 * hl + 65],
                        pt_tiles[pe][kt], start=(kt == 0), stop=(kt == 7))
                rcb = sb.tile([1, 512], BF16, tag="recipb", bufs=2,
                              name=f"rcb_{qb}_{p4}_{pe}")
                if tail:
                    # shortcut for the final groups (where the divide chain is
                    # exposed): stage the unnormalized ctx rows on the
                    # now-idle Act engine in parallel with the reciprocal
                    un = sb.tile([64, 512], F32, tag="bcs", bufs=2,
                                 name=f"un_{qb}_{p4}_{pe}")
                    nc.scalar.activation(un, ps_ctx[0:64, :], AF.Copy)
                with nc.allow_low_precision(reason="softmax denominators are "
                                            "O(100); bf16 reciprocal is ample"):
                    nc.vector.reciprocal(rcb, ps_ctx[64:65, :])
                nc.tensor.matmul(ps_ctx[64:128, :], ones1_sb[0:1, 0:64], rcb,
                                 start=True, stop=True)
                dst = ctxT_sb[64 * pe:64 * pe + 64, p4, qb * 512:qb * 512 + 512]
                if tail:
                    nc.vector.tensor_tensor(dst, un, ps_ctx[64:128, :], OP.mult)
                else:
                    bc_sb = sb.tile([64, 512], F32, tag="bcs", bufs=2,
                                    name=f"bcs_{qb}_{p4}_{pe}")
                    nc.vector.tensor_copy(bc_sb, ps_ctx[64:128, :])
                    nc.vector.tensor_tensor(dst, ps_ctx[0:64, :], bc_sb,
                                            OP.mult)

        def fc_unit(lb, last=False, use_st=False):
            # evictions split across Act (db0) and DVE (db1) so the final
            # units drain in parallel; out DMAs on Pool (SWDGE) except the
            # very last, which goes through the idle SP queue (HWDGE).
            # use_st: trailing units borrow the idle scores psum ring for db1
            # so they don't wait on cfc slots held by the final divide chains
            ost = sb.tile([128, 2, 512], BF16, tag="ostage", bufs=2,
                          name=f"ost_{lb}")
            for db in range(2):
                if use_st:
                    ps = psB.tile([128, 1024], F32, tag="st", bufs=2,
                                  name=f"fcs_{lb}_{db}")[:, 0:512]
                else:
                    ps = psB.tile([128, 512], F32, tag="cfc", bufs=4,
                                  name=f"fc_{lb}_{db}")
                for t4 in range(4):
                    nc.tensor.matmul(
                        ps, ctxT_sb[:, t4, lb * 128:lb * 128 + 128],
                        Wfc_sb[:, t4, db * 512:db * 512 + 512],
                        start=(t4 == 0), stop=(t4 == 3))
                if db == 0:
                    nc.scalar.activation(ost[:, db], ps, AF.Copy)
                else:
                    nc.vector.tensor_copy(ost[:, db], ps)
                if last:
                    eng = nc.gpsimd if db == 0 else nc.sync
                    eng.dma_start(odv[:, lb, db * 512:db * 512 + 512],
                                  ost[:, db])
            if not last:
                nc.gpsimd.dma_start(odv[:, lb, :],
                                    ost.rearrange("p db c -> p (db c)"))

        if lim >= 2:
            conv(sigq_sb, cwq_tiles, qT_sb, "q")
        if lim >= 3:
            conv(sigk_sb, cwk_tiles, kT_sb, "k")
        if lim >= 4:
            # the first vp units run on the psA ring so the psA->psB pool
            # boundary (which waits for psA quiescence) lands mid-vp, not
            # right after convk's eviction chain
            for lb in range(4 if lim >= 5 else 8):
                vp_unit(lb)

        psA.release()
        psB = tc.alloc_tile_pool(name="psB", bufs=1, space="PSUM")

        if lim >= 5:
            # the remaining vp units weave into the first two score groups to
            # cover the st-psum ring warmup; afterwards the steady pattern is
            # Sa(g) | C(g-1) | Sb(g) | fc-unit
            groups = [(qb, p4) for qb in range(2) for p4 in range(4)]
            pt0 = [[None] * 8 for _ in range(2)]
            pt1 = [[None] * 8 for _ in range(2)]
            vp_unit(4)
            scores_half(0, 0, 0, pt0)
            vp_unit(5)
            scores_half(0, 0, 1, pt0)
            vp_unit(6)
            scores_half(0, 1, 0, pt1)
            vp_unit(7)
            ctx_phase(0, 0, pt0)
            scores_half(0, 1, 1, pt1)
            pend = (0, 1, pt1)
            fc_ready = []  # lb units whose qb-half of ctxT is complete
            for qb, p4 in groups[2:]:
                pt_tiles = [[None] * 8 for _ in range(2)]
                scores_half(qb, p4, 0, pt_tiles)
                ctx_phase(*pend)
                if lim >= 6 and pend[1] == 3:
                    fc_ready.extend(range(4 * pend[0], 4 * pend[0] + 4))
                scores_half(qb, p4, 1, pt_tiles)
                if lim >= 6 and fc_ready:
                    fc_unit(fc_ready.pop(0))
                pend = (qb, p4, pt_tiles)
            if lim >= 6 and fc_ready:
                fc_unit(fc_ready.pop(0))
            ctx_phase(*pend, tail=True)
            if lim >= 6:
                if fc_ready:
                    fc_unit(fc_ready.pop(0))
                fc_ready.extend(range(4, 8))
                for lb in fc_ready:
                    fc_unit(lb, last=(lb == 7))

        psB.release()
        sb.release()
        dr.release()

    nc.finalize()
    return nc


# ----------------------------------------------------------------------------
# host-side data prep
# ----------------------------------------------------------------------------
def _host_prep(inp, flen, zero_bias):
    """Build the 8 per-core input dicts (core ci = 2*b + g)."""
    # per-parity shared tensors (g = 0, 1)
    shared = []
    for g in range(2):
        pi = np.arange(DM) ^ (512 * g)
        d = {}
        d["WQ"] = np.ascontiguousarray(inp["WQ"][:, pi]).astype(bf16)
        d["WK"] = np.ascontiguousarray(inp["WK"][:, pi]).astype(bf16)
        d["WV"] = np.ascontiguousarray(inp["WV"][:, 512 * g:512 * g + 512]).astype(bf16)
        d["Wfc"] = np.ascontiguousarray(inp["Wfc"][512 * g:512 * g + 512, :]).astype(bf16)
        if not zero_bias:
            bQ = inp["bQ"][pi].astype(np.float32)
            bK = inp["bK"][pi].astype(np.float32)
            bV = inp["bV"][512 * g:512 * g + 512].astype(np.float32)
            d["bQn"] = bQ[None, :].astype(bf16)
            d["bKn"] = bK[None, :].astype(bf16)
            d["bVn"] = bV[None, :].astype(bf16)
        for name, key in (("cwq", "conv_q"), ("cwk", "conv_k")):
            c = np.asarray(inp[key])[:, :, :flen].astype(np.float32)  # (d, o, f)
            c = np.ascontiguousarray(c.transpose(2, 0, 1))            # (f, d, o)
            c = c[:, pi, :][:, :, pi]
            # layout (8 dt, 128 p, 4 quarter, flen f, 256): column grouping so
            # each conv pass loads only its own o'-quarter of the weights
            c = c.transpose(1, 0, 2).reshape(8, 128, flen, 4, 256)
            c = np.ascontiguousarray(c.transpose(0, 1, 3, 2, 4)).reshape(8, 128, flen * 1024)
            d[name] = c.astype(bf16)
        em = np.zeros((128, 4), np.float32)
        em[:, :] = np.array([0, 0, 1, 1], np.float32) if g == 0 else \
            np.array([1, 1, 0, 0], np.float32)
        d["emask"] = em
        shared.append(d)

    maps = []
    for b in range(B):
        for g in range(2):
            sigma = np.arange(L) ^ (8 * g)
            m = dict(shared[g])
            m["Qt"] = np.ascontiguousarray(np.asarray(inp["Q"])[b][sigma, :].T).astype(bf16)
            m["Kt"] = np.ascontiguousarray(np.asarray(inp["K"])[b][sigma, :].T).astype(bf16)
            m["Vt"] = np.ascontiguousarray(np.asarray(inp["V"])[b][sigma, :].T).astype(bf16)
            maps.append(m)
    return maps


def _combine(results, inp):
    out = np.zeros((B, L, DM), np.float32)
    for b in range(B):
        for g in range(2):
            sigma = np.arange(L) ^ (8 * g)
            out[b] += np.asarray(results[2 * b + g]["out"]).astype(np.float32)[sigma, :]
        out[b] += np.asarray(inp["bfc"], dtype=np.float32)
    return out


def _get_program(flen, zero_bias=False):
    key = (flen, zero_bias)
    if key not in _CACHE:
        _CACHE[key] = _build(flen, zero_bias=zero_bias)
    return _CACHE[key]


def run_on_cores(inputs, trace=False):
    """Run the SPMD kernel; returns (full_output, BassKernelResults)."""
    from concourse.bass_utils import run_bass_kernel_spmd
    inp = {k: np.asarray(v) for k, v in inputs.items()}
    f_s = np.array(FILTER_LENGTHS, np.float32)
    flen = int(FILTER_LENGTHS[int(np.argmax(f_s * np.asarray(inp["w"], np.float32)))])
    zb = all(not np.any(np.asarray(inp[k])) for k in ("bQ", "bK", "bV"))
    nc = _get_program(flen, zero_bias=zb)
    in_maps = _host_prep(inp, flen, zb)
    res = run_bass_kernel_spmd(nc, in_maps, list(range(N_CORES)), trace=trace)
    return _combine(res.results, inp), res


def kernel(**inputs) -> np.ndarray:
    out, _ = run_on_cores(inputs, trace=False)
    return out
